# revision 7
# baseline (speedup 1.0000x reference)
"""TRN2 Bass kernel for nn_CrossAttention_71287867179098.

Cross attention: out = softmax((x1@Wq) @ (x2@Wk)^T / sqrt(d)) @ (x2@Wv)
Shapes: x_1 [4096,1024], x_2 [4096,1024], W_* [1024,1024], out [4096,1024], fp32.

Sharding: query rows (x_1) split across 8 cores (512 rows each); x_2 and
weights replicated. Each core runs one-pass flash attention over kv chunks.

Algebra: kv-side projections folded out so the 4096-length kv axis is touched
by exactly one matmul on each side of the softmax:
  scores = (Q @ Wk^T) @ x2^T       (G^T = Wk @ Q^T computed once)
  out    = ((P @ x2) @ Wv) / sums  (T accumulated in the flash loop)

Precision: PE matmuls run fp16 with 2-way hi/lo splits (A ~ Ah + Al;
A@B ~ Ah@Bh + Al@Bh + Ah@Bl) on the x1->Q->G->scores chain, which the
near-one-hot softmax requires. P@x2 and T@Wv use single fp16. fp32 PSUM.

Perf structure (vs the PE-transpose baseline):
- ALL transposes go through DMA (dma_start_transpose, XBAR 16x128 tiles,
  2-byte dtype) via fp16 DRAM roundtrips - zero PE transpose work, zero
  PSUM-evacuation traffic on scalar/vector. The PE stream is pure matmuls.
- All input DMAs issued up front on parallel rings; hi/lo splits are big
  natural-layout ops (scalar hi-cast, vector lo-subtract), streamed
  through small quarter-ring staging buffers to fit SBUF.
- Software pipelining: T(q) matmuls are emitted after scores(q+1) so the
  P^T DMA roundtrip hides; chunk prep runs one chunk ahead; phase-2 output
  blocks interleave into the last chunk. The PE never idles, which also
  keeps it at the 2.4GHz p-state (it drops toward 1.2GHz after any gap).
"""

import sys

sys.path.insert(0, "/opt/trn_rl_repo")

import numpy as np

import concourse.bass as bass
from concourse import bacc
import concourse.mybir as mybir
import concourse.tile as tile
from concourse.bass_utils import run_bass_kernel_spmd

F32 = mybir.dt.float32
F16 = mybir.dt.float16
AX = mybir.AxisListType
ALU = mybir.AluOpType
ACTF = mybir.ActivationFunctionType

P = 128
D = 1024          # d_in == d_kq == d_v
CO = D // P       # contraction chunks (8)
NQ = 512          # query rows per core
QT = NQ // P      # query tiles per core (4)
NKV = 4096
CHUNK = 512       # kv rows per chunk
NCH = NKV // CHUNK
JO = CHUNK // P   # kv subtiles per chunk (4)
NCORES = 8
INV_SQRT_D = 1.0 / 32.0


def build_kernel() -> bass.Bass:
    nc = bacc.Bacc(target_bir_lowering=False)
    x1_d = nc.dram_tensor("x1s", [NQ, D], F32, kind="ExternalInput")
    x2_d = nc.dram_tensor("x2", [NKV, D], F32, kind="ExternalInput")
    wq_d = nc.dram_tensor("Wq", [D, D], F32, kind="ExternalInput")
    wk_d = nc.dram_tensor("Wk", [D, D], F32, kind="ExternalInput")
    wv_d = nc.dram_tensor("Wv", [D, D], F32, kind="ExternalInput")
    out_d = nc.dram_tensor("out", [NQ, D], F32, kind="ExternalOutput")

    # fp16 DRAM scratch for DMA-transpose roundtrips (unique slots, no WAR)
    x1h_d = nc.dram_tensor("x1h", [NQ, D], F16, kind="Internal")
    x1l_d = nc.dram_tensor("x1l", [NQ, D], F16, kind="Internal")
    wkh_d = nc.dram_tensor("wkh", [D, D], F16, kind="Internal")
    wkl_d = nc.dram_tensor("wkl", [D, D], F16, kind="Internal")
    x2h_d = nc.dram_tensor("x2h", [NKV, D], F16, kind="Internal")
    x2l_d = nc.dram_tensor("x2l", [NKV, D], F16, kind="Internal")
    p_d = nc.dram_tensor("pd", [NCH, QT, P, CHUNK], F16, kind="Internal")
    tn_d = nc.dram_tensor("tnd", [QT, P, D], F16, kind="Internal")

    with tile.TileContext(nc) as tc:
        with (
            tc.tile_pool(name="persist", bufs=1) as persist,
            tc.tile_pool(name="stats", bufs=8) as stats,
            tc.tile_pool(name="psQG", bufs=2, space="PSUM") as psQG,
            tc.tile_pool(name="psS", bufs=2, space="PSUM") as psS,
            tc.tile_pool(name="psO", bufs=2, space="PSUM") as psO,
            tc.tile_pool(name="x2cq", bufs=4) as x2cq,      # fp32 jo quarters
            tc.tile_pool(name="x2np", bufs=2) as x2np,      # fp16 hi (T rhs)
            tc.tile_pool(name="x2lq", bufs=2) as x2lq,      # fp16 lo quarters
        ):
            # ---- persistent operands ----
            gt_h = persist.tile([P, CO, NQ], F16)   # G^T hi
            gt_l = persist.tile([P, CO, NQ], F16)   # G^T lo
            t_acc = [persist.tile([P, D], F32, name=f"t_acc{q}") for q in range(QT)]
            m_cur = [stats.tile([P, 1], F32, tag="m", name=f"m0_{q}") for q in range(QT)]
            s_cur = [stats.tile([P, 1], F32, tag="s", name=f"s0_{q}") for q in range(QT)]
            for q in range(QT):
                nc.gpsimd.memset(t_acc[q][:], 0.0)
                nc.gpsimd.memset(m_cur[q][:], -1e30)
                nc.gpsimd.memset(s_cur[q][:], 0.0)

            # ---- x2 chunk prep (jo-granular, DMA roundtrip transpose) ----
            def prep_load(t):
                qs = []
                for jo in range(JO):
                    r0 = t * CHUNK + jo * P
                    xq = x2cq.tile([P, D], F32, tag="x2c", name=f"x2c_{t}_{jo}")
                    nc.sync.dma_start(xq[:], x2_d[r0:r0 + P, :])
                    qs.append(xq)
                return qs

            def prep_split(t, quarters):
                x2n = x2np.tile([P, JO, D], F16, tag="x2n", name=f"x2n_{t}")
                for jo in range(JO):
                    xq = quarters[jo]
                    nc.scalar.activation(x2n[:, jo, :], xq[:], ACTF.Copy)
                    xl = x2lq.tile([P, D], F16, tag="x2l", name=f"x2l_{t}_{jo}")
                    nc.vector.scalar_tensor_tensor(
                        xl[:], xq[:], 1.0, x2n[:, jo, :], ALU.mult, ALU.subtract
                    )
                    r0 = t * CHUNK + jo * P
                    nc.sync.dma_start(x2l_d[r0:r0 + P, :], xl[:])
                sl = slice(t * CHUNK, (t + 1) * CHUNK)
                nc.sync.dma_start(
                    x2h_d[sl, :].rearrange("(jo p) c -> p jo c", p=P), x2n[:]
                )
                return x2n

            # ================= phase 0 =================
            with tc.tile_pool(name="pB", bufs=1) as pB:       # qt (Q^T..G^T)
                pA_ctx = tc.tile_pool(name="pA", bufs=1)      # x1t, wq_h/l
                pA = pA_ctx.__enter__()
                with (
                    tc.tile_pool(name="r32", bufs=2) as r32,
                    tc.tile_pool(name="r16", bufs=2) as r16,
                ):
                    # -- input DMAs + quarter-streamed splits --
                    x1_r = x1_d.rearrange("(io p) c -> p io c", p=P)
                    x1h_r = x1h_d.rearrange("(io p) c -> p io c", p=P)
                    x1l_r = x1l_d.rearrange("(io p) c -> p io c", p=P)
                    wq_r = wq_d.rearrange("(co p) d -> p co d", p=P)
                    wk_r = wk_d.rearrange("(co p) d -> p co d", p=P)
                    wkh_r = wkh_d.rearrange("(co p) d -> p co d", p=P)
                    wkl_r = wkl_d.rearrange("(co p) d -> p co d", p=P)

                    x1q, wqq, wkq = [], [], []
                    for w in range(4):
                        ws = slice(w * 256, (w + 1) * 256)
                        xq = r32.tile([P, QT, 256], F32, tag="x1q", name=f"x1q{w}")
                        nc.sync.dma_start(xq[:], x1_r[:, :, ws])
                        x1q.append(xq)
                    for w in range(4):
                        ws = slice(w * 256, (w + 1) * 256)
                        wf = r32.tile([P, CO, 256], F32, tag="wqq", name=f"wqq{w}")
                        nc.sync.dma_start(wf[:], wq_r[:, :, ws])
                        wqq.append(wf)
                    for w in range(4):
                        ws = slice(w * 256, (w + 1) * 256)
                        wf = r32.tile([P, CO, 256], F32, tag="wkq", name=f"wkq{w}")
                        nc.sync.dma_start(wf[:], wk_r[:, :, ws])
                        wkq.append(wf)
                    c0q = prep_load(0)
                    c1q = prep_load(1)

                    # x1 quarters: split, write fp16 hi/lo to DRAM
                    for w in range(4):
                        ws = slice(w * 256, (w + 1) * 256)
                        xh = r16.tile([P, QT, 256], F16, tag="x1nh", name=f"x1nh{w}")
                        nc.scalar.activation(xh[:], x1q[w][:], ACTF.Copy)
                        nc.sync.dma_start(x1h_r[:, :, ws], xh[:])
                        xl = r16.tile([P, QT, 256], F16, tag="x1nl", name=f"x1nl{w}")
                        nc.vector.scalar_tensor_tensor(
                            xl[:], x1q[w][:], 1.0, xh[:], ALU.mult, ALU.subtract
                        )
                        nc.sync.dma_start(x1l_r[:, :, ws], xl[:])

                    # x1T hi/lo [c, i] via DMA transpose
                    x1t_h = pA.tile([P, CO, NQ], F16, tag="x1th")
                    nc.sync.dma_start_transpose(x1t_h[:], x1h_d[:, :])
                    x1t_l = pA.tile([P, CO, NQ], F16, tag="x1tl")
                    nc.sync.dma_start_transpose(x1t_l[:], x1l_d[:, :])

                    # Wq quarters -> full fp16 hi/lo (natural lhsT, no xpose)
                    wq_h = pA.tile([P, CO, D], F16, tag="wqh")
                    wq_l = pA.tile([P, CO, D], F16, tag="wql")
                    for w in range(4):
                        ws = slice(w * 256, (w + 1) * 256)
                        nc.scalar.activation(
                            wq_h[:, :, ws], wqq[w][:], ACTF.Copy
                        )
                        nc.vector.scalar_tensor_tensor(
                            wq_l[:, :, ws], wqq[w][:], 1.0, wq_h[:, :, ws],
                            ALU.mult, ALU.subtract,
                        )

                    # Wk quarters: split, write fp16 hi/lo to DRAM
                    for w in range(4):
                        ws = slice(w * 256, (w + 1) * 256)
                        wh = r16.tile([P, CO, 256], F16, tag="wkh", name=f"wkh{w}")
                        nc.scalar.activation(wh[:], wkq[w][:], ACTF.Copy)
                        nc.sync.dma_start(wkh_r[:, :, ws], wh[:])
                        wl = r16.tile([P, CO, 256], F16, tag="wkl", name=f"wkl{w}")
                        nc.vector.scalar_tensor_tensor(
                            wl[:], wkq[w][:], 1.0, wh[:], ALU.mult, ALU.subtract
                        )
                        nc.sync.dma_start(wkl_r[:, :, ws], wl[:])

                # -- Q^T [d, i]: lhsT=Wq[c,d] tiles, rhs=x1T[c,i] --
                qt_h = pB.tile([P, CO, NQ], F16, tag="qth")
                qt_l = pB.tile([P, CO, NQ], F16, tag="qtl")
                for dc in range(CO):
                    ps = psQG.tile([P, NQ], F32, tag="ps")
                    n = 0
                    for wt, xt in ((wq_h, x1t_h), (wq_l, x1t_h), (wq_h, x1t_l)):
                        for co in range(CO):
                            nc.tensor.matmul(
                                ps[:],
                                wt[:, co, dc * P:(dc + 1) * P],
                                xt[:, co, :],
                                start=(n == 0),
                                stop=(n == 23),
                            )
                            n += 1
                    nc.scalar.activation(qt_h[:, dc, :], ps[:], ACTF.Copy)
                    nc.vector.scalar_tensor_tensor(
                        qt_l[:, dc, :], ps[:], 1.0, qt_h[:, dc, :],
                        ALU.mult, ALU.subtract,
                    )
                    # x2 chunk 0/1 splits, interleaved with Q^T drains
                    if dc == 1:
                        x2n0 = prep_split(0, c0q)
                    elif dc == 3:
                        x2n1 = prep_split(1, c1q)
                pA_ctx.__exit__(None, None, None)   # free x1t, wq_h/l

                # -- G^T [c, i] = Wk @ Q^T: lhsT=WkT[d,c] tiles, rhs=QT[d,i] --
                with tc.tile_pool(name="pC", bufs=1) as pC:
                    wkt_h = pC.tile([P, CO, D], F16, tag="wkth")
                    nc.sync.dma_start_transpose(wkt_h[:], wkh_d[:, :])
                    wkt_l = pC.tile([P, CO, D], F16, tag="wktl")
                    nc.sync.dma_start_transpose(wkt_l[:], wkl_d[:, :])
                    for cc in range(CO):
                        ps = psQG.tile([P, NQ], F32, tag="ps")
                        n = 0
                        for wt, qa in ((wkt_h, qt_h), (wkt_l, qt_h), (wkt_h, qt_l)):
                            for dc in range(CO):
                                nc.tensor.matmul(
                                    ps[:],
                                    wt[:, dc, cc * P:(cc + 1) * P],
                                    qa[:, dc, :],
                                    start=(n == 0),
                                    stop=(n == 23),
                                )
                                n += 1
                        nc.scalar.activation(gt_h[:, cc, :], ps[:], ACTF.Copy)
                        nc.vector.scalar_tensor_tensor(
                            gt_l[:, cc, :], ps[:], 1.0, gt_h[:, cc, :],
                            ALU.mult, ALU.subtract,
                        )

            # ================= flash loop =================
            with (
                tc.tile_pool(name="x2tp", bufs=2) as x2tp,
                tc.tile_pool(name="wvp", bufs=1) as wvp,
                tc.tile_pool(name="wvr", bufs=2) as wvr,
                tc.tile_pool(name="pp", bufs=2) as ppool,
                tc.tile_pool(name="outp", bufs=2) as outp,
            ):
                def prep_xpose(t):
                    sl = slice(t * CHUNK, (t + 1) * CHUNK)
                    x2t_h = x2tp.tile(
                        [P, CO, CHUNK], F16, tag="x2th", name=f"x2th_{t}"
                    )
                    nc.sync.dma_start_transpose(x2t_h[:], x2h_d[sl, :])
                    x2t_l = x2tp.tile(
                        [P, CO, CHUNK], F16, tag="x2tl", name=f"x2tl_{t}"
                    )
                    nc.sync.dma_start_transpose(x2t_l[:], x2l_d[sl, :])
                    return x2t_h, x2t_l

                wv_h = wvp.tile([P, CO, D], F16)
                wv_r = wv_d.rearrange("(co p) d -> p co d", p=P)

                def wv_piece(w):
                    ws = slice(w * 256, (w + 1) * 256)
                    wf = wvr.tile([P, CO, 256], F32, tag="wvf", name=f"wvf{w}")
                    nc.sync.dma_start(wf[:], wv_r[:, :, ws])
                    nc.scalar.activation(wv_h[:, :, ws], wf[:], ACTF.Copy)

                cur = {0: (x2n0,) + prep_xpose(0), 1: (x2n1,) + prep_xpose(1)}
                pend_T = None   # (t, q, x2n, p_t, fsc)
                ph2_done = set()

                def emit_T(tt_, qq, x2n_t, p_t, fsc):
                    for dh in range(2):
                        ps_o = psO.tile([P, 512], F32, tag="ps_o")
                        for jt in range(JO):
                            nc.tensor.matmul(
                                ps_o[:],
                                p_t[:, jt, :],
                                x2n_t[:, jt, dh * 512:(dh + 1) * 512],
                                start=(jt == 0),
                                stop=(jt == JO - 1),
                            )
                        dst = t_acc[qq][:, dh * 512:(dh + 1) * 512]
                        nc.vector.scalar_tensor_tensor(
                            dst, dst, fsc[:], ps_o[:], ALU.mult, ALU.add
                        )

                out_ap = out_d.rearrange("(qo p) d -> p qo d", p=P)

                def emit_phase2(q):
                    # normalize, tt = (T/s)^T via DMA, O = tt-chain @ Wv
                    ph2_done.add(q)
                    rcp = stats.tile([P, 1], F32, tag="rcp")
                    nc.vector.reciprocal(rcp[:], s_cur[q][:])
                    tn = outp.tile([P, D], F16, tag="tn")
                    nc.scalar.activation(
                        tn[:], t_acc[q][:], ACTF.Copy, scale=rcp[:]
                    )
                    nc.sync.dma_start(tn_d[q], tn[:])
                    tt = outp.tile([P, CO, P], F16, tag="tt")
                    nc.sync.dma_start_transpose(tt[:], tn_d[q])
                    o_sb = outp.tile([P, D], F32, tag="osb")
                    for dh in range(2):
                        ps = psQG.tile([P, 512], F32, tag="ps")
                        for cc in range(CO):
                            nc.tensor.matmul(
                                ps[:],
                                tt[:, cc, :],
                                wv_h[:, cc, dh * 512:(dh + 1) * 512],
                                start=(cc == 0),
                                stop=(cc == CO - 1),
                            )
                        nc.vector.tensor_copy(
                            o_sb[:, dh * 512:(dh + 1) * 512], ps[:]
                        )
                    nc.sync.dma_start(out_ap[:, q, :], o_sb[:])

                for t in range(NCH):
                    x2n_t, x2t_h, x2t_l = cur.pop(t)
                    for q in range(QT):
                        # scores for (t, q)
                        ps_s = psS.tile([P, CHUNK], F32, tag="ps_s")
                        n = 0
                        for ga, xa in (
                            (gt_h, x2t_h), (gt_l, x2t_h), (gt_h, x2t_l)
                        ):
                            for cc in range(CO):
                                nc.tensor.matmul(
                                    ps_s[:],
                                    ga[:, cc, q * P:(q + 1) * P],
                                    xa[:, cc, :],
                                    start=(n == 0),
                                    stop=(n == 23),
                                )
                                n += 1

                        # T-update of the previous q-iteration (its P^T
                        # roundtrip hid under this q's scores matmuls)
                        if pend_T is not None:
                            emit_T(*pend_T)
                            pend_T = None
                        # interleave phase-2 blocks into the last chunk
                        if t == NCH - 1 and q >= 2:
                            emit_phase2(q - 2)

                        # online softmax stats
                        rm = stats.tile([P, 1], F32, tag="rm")
                        nc.vector.reduce_max(rm[:], ps_s[:], axis=AX.X)
                        m_new = stats.tile([P, 1], F32, tag="m")
                        nc.vector.tensor_tensor(
                            m_new[:], m_cur[q][:], rm[:], ALU.max
                        )
                        bias = stats.tile([P, 1], F32, tag="bias")
                        nc.vector.tensor_scalar_mul(bias[:], m_new[:], -INV_SQRT_D)
                        fsc = stats.tile([P, 1], F32, tag="fsc")
                        nc.scalar.activation(
                            fsc[:], m_cur[q][:], ACTF.Exp,
                            bias=bias[:], scale=INV_SQRT_D,
                        )
                        p_c = ppool.tile([P, CHUNK], F16, tag="pc")
                        rs = stats.tile([P, 1], F32, tag="rs")
                        nc.scalar.activation(
                            p_c[:], ps_s[:], ACTF.Exp,
                            bias=bias[:], scale=INV_SQRT_D, accum_out=rs[:],
                        )
                        s_new = stats.tile([P, 1], F32, tag="s")
                        nc.vector.scalar_tensor_tensor(
                            s_new[:], s_cur[q][:], fsc[:], rs[:],
                            ALU.mult, ALU.add,
                        )
                        m_cur[q] = m_new
                        s_cur[q] = s_new

                        # P^T via DMA roundtrip
                        nc.sync.dma_start(p_d[t, q], p_c[:])
                        p_t = ppool.tile([P, JO, P], F16, tag="pt")
                        nc.sync.dma_start_transpose(p_t[:], p_d[t, q])
                        pend_T = (t, q, x2n_t, p_t, fsc)

                        # next-chunk prep hooks (one chunk of lead time)
                        if q == 0 and 2 <= t + 1 < NCH:
                            cur[t + 1] = [prep_split(t + 1, cur[t + 1])]
                        elif q == 1 and 2 <= t + 1 < NCH:
                            cur[t + 1] = tuple(cur[t + 1]) + prep_xpose(t + 1)
                        elif q == 2 and t + 2 < NCH:
                            cur[t + 2] = prep_load(t + 2)
                        if t == 0:
                            wv_piece(q)

                # tail: last T-update + remaining phase-2 blocks
                if pend_T is not None:
                    emit_T(*pend_T)
                for q in range(QT):
                    if q not in ph2_done:
                        emit_phase2(q)

    nc.compile()
    return nc


_NC_CACHE = None


def _get_nc():
    global _NC_CACHE
    if _NC_CACHE is None:
        _NC_CACHE = build_kernel()
    return _NC_CACHE


def _run(inputs, trace=False):
    """Returns (output [4096,1024] f32, exec_time_ns or None, results obj)."""
    x1 = np.ascontiguousarray(np.asarray(inputs["x_1"], dtype=np.float32))
    x2 = np.ascontiguousarray(np.asarray(inputs["x_2"], dtype=np.float32))
    wq = np.ascontiguousarray(np.asarray(inputs["W_query"], dtype=np.float32))
    wk = np.ascontiguousarray(np.asarray(inputs["W_key"], dtype=np.float32))
    wv = np.ascontiguousarray(np.asarray(inputs["W_value"], dtype=np.float32))

    nc = _get_nc()
    in_maps = [
        {
            "x1s": x1[c * NQ:(c + 1) * NQ],
            "x2": x2,
            "Wq": wq,
            "Wk": wk,
            "Wv": wv,
        }
        for c in range(NCORES)
    ]
    br = run_bass_kernel_spmd(nc, in_maps, list(range(NCORES)), trace=trace)
    out = np.concatenate([br.results[c]["out"] for c in range(NCORES)], axis=0)
    return out.astype(np.float32), br.exec_time_ns, br


def kernel(**inputs) -> np.ndarray:
    out, _, _ = _run(inputs)
    return out
